# revision 13
# baseline (speedup 1.0000x reference)
"""Trainium2 Bass kernel for EnhancedGATModel (3-layer GATv2, N=50000, E=800000).

v2 design (8 cores, dst-partitioned graph):
- att magnitudes folded into Wl/Wr columns (host), columns permuted
  positive-att-first per head: per-edge logits become prefix-sum
  differences of leaky(xl''+xr'') -- one custom DVE scan op per 8-tile
  group, no per-edge multiply by att, no per-edge reduce.
- per-edge xr'' comes from a PE one-hot broadcast matmul out of an
  SBUF-resident per-block xr table (no xr gather).
- messages scattered TRANSPOSED ([channel, dst]) by PE with fp8 host
  one-hots; softmax denominator via a tiny per-tile matmul; per-dst
  reciprocal expanded back to [channel, dst] with a head-mask matmul.
- BN+relu fused into one ScalarE activation per 128-channel chunk
  (per-partition scale/bias absorb 1/|att| and the permutation).
- residual handled by a split matmul at the L2 node phase (h2+h1 both
  feed L2 tables) -- no elementwise residual add.
- bf16 tables/gathers, fp8 one-hots, 4 SWDGE queues for gather overlap.
"""
import sys
import numpy as np

sys.path.insert(0, "/opt/trn_rl_repo")

import ml_dtypes
import concourse.bass as bass
import concourse.mybir as mybir
import concourse.tile as tile
from concourse import bacc
from concourse import dve_ops as _dve_ops
from concourse.bass_utils import run_bass_kernel_spmd
from concourse.dve_spec import (
    Spec, Scan, Src0, Src1, C0, AluOp as SAlu, lower as dve_lower, maxx,
)
from concourse.dve_ops import DveOp
from concourse.dve_uop import DveOpSpec

F32 = mybir.dt.float32
BF16 = mybir.dt.bfloat16
F8 = mybir.dt.float8e4
I16 = mybir.dt.int16
AF = mybir.ActivationFunctionType
ALU = mybir.AluOpType
NPBF = ml_dtypes.bfloat16
NPF8 = ml_dtypes.float8_e4m3

N = 50000
NCORES = 8
NPC = N // NCORES          # 6250
BLOCK = 128
NBLK = (NPC + BLOCK - 1) // BLOCK   # 49
HALF = N // 2
D_IN, HID, HEADS, OUT = 128, 64, 4, 2
HC = HEADS * HID           # 256
NEG = 0.2
BN_EPS = 1e-5
G = 4                      # tiles per scan group (PSUM: [128, G, 256] f32 = 2 banks)
GMAX = 8                   # dma_gather <=1024 idxs per op


# ------------------------------------------------------------- custom DVE op
def _register_scan_op():
    """out = inclusive prefix sum (along free dims) of leaky_relu(in0+in1),
    slope s0."""
    name = "GAT_LEAKY_PREFIX_ANT"
    if name in _dve_ops._SUB_OPCODE_FOR_NAME:
        return next(o for o in _dve_ops.OPS if o.name == name)
    z = Src0 + Src1
    body = Scan(SAlu.ADD, maxx(z, z * C0))

    def _ref(in0, in1, s0, s1, imm2):
        zz = in0.astype(np.float32) + in1.astype(np.float32)
        lk = np.maximum(zz, zz * np.asarray(s0, np.float32).reshape(-1, 1)
                        if isinstance(s0, np.ndarray) else zz * s0)
        P = in0.shape[0]
        return np.add.accumulate(lk.reshape(P, -1), axis=1).reshape(in0.shape)

    spec = Spec(body=body, reference=_ref)
    shas = {}
    for ver in ("v3",):
        uops = dve_lower(spec, ver=ver)
        shas[ver] = DveOpSpec(name=name, opcode=0, uops=uops, rd1_en=True).sha(ver)
    op = DveOp(name, spec, subdim=False, uops_sha=shas)
    row = max(_dve_ops._SUB_OPCODE_FOR_NAME.values()) + 1
    assert row < 0x20
    _dve_ops.OPS.append(op)
    _dve_ops._SUB_OPCODE_FOR_NAME[name] = row
    _dve_ops.CUSTOM_DVE_SPECS[name] = spec
    return op


SCAN_OP = _register_scan_op()


# ---------------------------------------------------------------- host fold
def fold_att(att):
    att = np.asarray(att, np.float64)
    H, C = att.shape
    lam = np.abs(att)
    lam = np.maximum(lam, 1e-4 * max(lam.mean(), 1e-30))
    perm = np.zeros(H * C, np.int64)
    k = np.zeros(H, np.int64)
    for h in range(H):
        pos = np.where(att[h] >= 0)[0]
        neg = np.where(att[h] < 0)[0]
        k[h] = len(pos)
        perm[h * C:(h + 1) * C] = h * C + np.concatenate([pos, neg])
    return lam.reshape(-1), perm, k


def prep_weights(ip):
    f = {}
    lam0, p0, k0 = fold_att(np.asarray(ip["att0"], np.float64))
    lam1, p1, k1 = fold_att(np.asarray(ip["att1"], np.float64))
    lam2, p2, k2 = fold_att(np.asarray(ip["att2"], np.float64))
    W = lambda nm: np.asarray(ip[nm], np.float64)
    f["Wl0f"] = (W("Wl0") * lam0[None, :])[:, p0]
    f["Wr0f"] = (W("Wr0") * lam0[None, :])[:, p0]
    f["Wl1f"] = (W("Wl1") * lam1[None, :])[p0][:, p1]
    f["Wr1f"] = (W("Wr1") * lam1[None, :])[p0][:, p1]
    f["Wl2f_h2"] = (W("Wl2") * lam2[None, :])[p1][:, p2]
    f["Wl2f_h1"] = (W("Wl2") * lam2[None, :])[p0][:, p2]
    f["Wr2f_h2"] = (W("Wr2") * lam2[None, :])[p1][:, p2]
    f["Wr2f_h1"] = (W("Wr2") * lam2[None, :])[p0][:, p2]
    g = np.asarray(ip["bn_gamma"], np.float64); bt = np.asarray(ip["bn_beta"], np.float64)
    mu = np.asarray(ip["bn_mean"], np.float64); var = np.asarray(ip["bn_var"], np.float64)
    for l, (perm, lam) in enumerate(((p0, lam0), (p1, lam1))):
        a = g[l] / np.sqrt(var[l] + BN_EPS)
        b = bt[l] - mu[l] * a + a * np.asarray(ip[f"bias{l}"], np.float64)
        f[f"aT{l}"] = a[perm] / lam[perm]
        f[f"bT{l}"] = b[perm]
    f["rho2"] = np.argsort(p2)
    f["inv_lam2"] = 1.0 / lam2
    bias2 = np.asarray(ip["bias2"], np.float64)
    f["db2"] = float(bias2[1] - bias2[0])
    f["k0"], f["k1"], f["k2"] = k0, k1, int(k2[0])
    f["W_in"] = np.asarray(ip["W_in"], np.float64)
    f["b_in"] = np.asarray(ip["b_in"], np.float64)
    return f


# ---------------------------------------------------------------- host prep
def _balance_nodes(src_all, dst_all):
    """Assign nodes to (core, block) bins so that every bin's lo-half and
    hi-half edge loads are both balanced (the schedule pads each (block,
    half) to the max over cores).  Phase 1 splits nodes into the two table
    halves (cores 0-3 vs 4-7) by degree; phase 2 deals nodes into bins of
    their half, balancing the then-known per-half in-edge loads."""
    import heapq
    deg = np.bincount(dst_all, minlength=N)
    order = np.argsort(-deg, kind="stable")
    # phase 1: alternate nodes (by degree) between the two halves
    in_lo = np.zeros(N, bool)
    in_lo[order[0::2]] = True
    if in_lo.sum() != HALF // NPC * NPC // 1 and in_lo.sum() != N // 2:
        pass
    # force exact N/2 membership
    lo_ids = order[0::2][:N // 2]
    in_lo[:] = False
    in_lo[lo_ids] = True
    if in_lo.sum() < N // 2:
        extra = np.where(~in_lo)[0][:N // 2 - in_lo.sum()]
        in_lo[extra] = True
    # per-node per-half in-degree (src half known now)
    lo_d = np.bincount(dst_all[in_lo[src_all]], minlength=N)
    hi_d = deg - lo_d
    # phase 2: deal within each half: 4 cores x NBLK bins
    node_list = [None] * NCORES
    for half, cores in ((True, (0, 1, 2, 3)), (False, (4, 5, 6, 7))):
        ids = np.where(in_lo == half)[0]
        ids = ids[np.argsort(-(deg[ids]), kind="stable")]
        cap = {(c, b): (BLOCK if b < NBLK - 1 else NPC - (NBLK - 1) * BLOCK)
               for c in cores for b in range(NBLK)}
        fill = {k: 0 for k in cap}
        nodes = {k: [] for k in cap}
        heap = [(0, 0, c, b) for c in cores for b in range(NBLK)]
        heapq.heapify(heap)
        for nd in ids:
            while True:
                mx, mn, c, b = heapq.heappop(heap)
                if fill[(c, b)] < cap[(c, b)]:
                    break
            nodes[(c, b)].append(nd)
            fill[(c, b)] += 1
            lo2 = -mn if False else 0
            # track (lo_load, hi_load) via encoded tuple
            # recompute loads stored alongside:
            nodes.setdefault((c, b, "lo"), 0)
            nodes.setdefault((c, b, "hi"), 0)
            nodes[(c, b, "lo")] += int(lo_d[nd])
            nodes[(c, b, "hi")] += int(hi_d[nd])
            if fill[(c, b)] < cap[(c, b)]:
                l_, h_ = nodes[(c, b, "lo")], nodes[(c, b, "hi")]
                heapq.heappush(heap, (max(l_, h_), min(l_, h_), c, b))
        for c in cores:
            # order bins so heavy bins share a block index across cores
            # (schedule pads each index to the max over cores); the partial
            # bin must stay at index NBLK-1 (device uses nreal there).
            full = sorted(range(NBLK - 1),
                          key=lambda b: max(nodes[(c, b, "lo")],
                                            nodes[(c, b, "hi")]))
            border = full + [NBLK - 1]
            node_list[c] = np.array(
                sum((nodes[(c, b)] for b in border), []), np.int64)
    pos = np.zeros(N, np.int64)
    for c in range(NCORES):
        pos[node_list[c]] = c * NPC + np.arange(NPC)
    return node_list, pos


def preprocess(edge_index):
    """Edge bucketing with degree-balanced node placement.

    idx  [NCORES, 128, 8*TT] int16  gather indices (wrapped layout)
    oh8  [NCORES, 128, TT*128] f8   scatter one-hot, row=edge, col=dst
    oh28 [NCORES, 128, TT*128] f8   bcast one-hot, row=dst, col=edge
    blk_runs: per block list of (tg, half, T); uniform across cores.
    """
    src0 = np.concatenate([edge_index[0], np.arange(N)]).astype(np.int64)
    dst0 = np.concatenate([edge_index[1], np.arange(N)]).astype(np.int64)
    node_list, pos = _balance_nodes(src0, dst0)
    src = pos[src0]
    dst = pos[dst0]
    order = np.argsort(dst, kind="stable")
    src, dst = src[order], dst[order]
    core_of = dst // NPC
    groups = {}
    for c in range(NCORES):
        m = core_of == c
        sc, dc = src[m], dst[m] - c * NPC
        blk = dc // BLOCK
        for b in range(NBLK):
            mb = blk == b
            sb_, db_ = sc[mb], dc[mb] % BLOCK
            lo = sb_ < HALF
            groups[(c, b, 0)] = (sb_[lo], db_[lo])
            groups[(c, b, 1)] = (sb_[~lo] - HALF, db_[~lo])
    blk_runs = []
    TT = 0
    for b in range(NBLK):
        runs = []
        for h in (0, 1):
            mx = max(len(groups[(c, b, h)][0]) for c in range(NCORES))
            T = (mx + 127) // 128
            if T > 0:
                runs.append((TT, h, T))
                TT += T
        blk_runs.append(runs)
    Tmax = max(sum(T for _, _, T in runs) for runs in blk_runs)
    Trun = max(T for runs in blk_runs for _, _, T in runs)

    idx = np.zeros((NCORES, 128, 8 * TT), np.int16)
    dstl = np.full((NCORES, TT, 128), -1, np.int64)
    for b in range(NBLK):
        for (tg, h, T) in blk_runs[b]:
            for c in range(NCORES):
                s, dl = groups[(c, b, h)]
                ne = len(s)
                pad = T * 128 - ne
                sp = np.concatenate([s, np.zeros(pad, np.int64)])
                wrap = sp.reshape(8 * T, 16).T.astype(np.int16)
                idx[c, :, 8 * tg:8 * (tg + T)] = np.tile(wrap, (8, 1))
                dfull = np.concatenate([dl, np.full(pad, -1, np.int64)])
                dstl[c, tg:tg + T] = dfull.reshape(T, 128)
    # one-hots, built directly as fp8 bytes (1.0 = 0x38 in e4m3)
    oh = np.zeros((NCORES, 128, TT, 128), np.uint8)
    oh2 = np.zeros((NCORES, 128, TT, 128), np.uint8)
    ci, ti, ei = np.where(dstl >= 0)
    dv = dstl[ci, ti, ei]
    oh[ci, ei, ti, dv] = 0x38
    oh2[ci, dv, ti, ei] = 0x38
    oh8 = oh.reshape(NCORES, 128, TT * 128).view(NPF8)
    oh28 = oh2.reshape(NCORES, 128, TT * 128).view(NPF8)
    return idx, oh8, oh28, blk_runs, TT, Tmax, Trun, node_list


def pack_consts(f):
    """Two packed const tensors: CB [128, *] bf16 and CF [128, *] f32."""
    bcols, bparts = {}, []
    fcols, fparts = {}, []

    def addb(name, arr):
        a = np.zeros((128, arr.shape[1]), np.float64)
        a[:arr.shape[0]] = arr
        bcols[name] = (arr.shape[0], sum(p.shape[1] for p in bparts), arr.shape[1])
        bparts.append(a)

    def addf(name, arr):
        a = np.zeros((128, arr.shape[1]), np.float64)
        a[:arr.shape[0]] = arr
        fcols[name] = (arr.shape[0], sum(p.shape[1] for p in fparts), arr.shape[1])
        fparts.append(a)

    addb("W_in", f["W_in"])                      # [128, 64]
    addb("Wl0f", f["Wl0f"]); addb("Wr0f", f["Wr0f"])          # [64, 256]
    addb("Wl1f0", f["Wl1f"][:128]); addb("Wl1f1", f["Wl1f"][128:])
    addb("Wr1f0", f["Wr1f"][:128]); addb("Wr1f1", f["Wr1f"][128:])
    for nm in ("Wl2f_h2", "Wl2f_h1", "Wr2f_h2", "Wr2f_h1"):
        addb(nm + "k0", f[nm][:128]); addb(nm + "k1", f[nm][128:])

    for l in (0, 1):
        addb(f"aT{l}", np.broadcast_to(f[f"aT{l}"][None, :], (128, HC)))
        addb(f"bT{l}", np.broadcast_to(f[f"bT{l}"][None, :], (128, HC)))
    addb("ident", np.eye(128))
    addf("b_in", f["b_in"].reshape(-1, 1))       # [64, 1]

    CB = np.concatenate(bparts, axis=1).astype(NPBF)
    CF = np.concatenate(fparts, axis=1).astype(np.float32)
    return CB, bcols, CF, fcols


# ---------------------------------------------------------------- device
def build(blk_runs, TT, Tmax, Trun, CBw, CFw, BCOLS, FCOLS, K0, K1, K2,
          RHO2, INV_LAM2, DB2):
    nc = bacc.Bacc("TRN2", target_bir_lowering=False, debug=False,
                   num_swdge_queues=4)

    xT = nc.dram_tensor("xT", [D_IN, NPC], BF16, kind="ExternalInput")
    idx = nc.dram_tensor("idx", [128, 8 * TT], I16, kind="ExternalInput")
    oh8 = nc.dram_tensor("oh8", [128, TT * 128], F8, kind="ExternalInput")
    oh28 = nc.dram_tensor("oh28", [128, TT * 128], F8, kind="ExternalInput")
    CB = nc.dram_tensor("CB", [128, CBw], BF16, kind="ExternalInput")
    CF = nc.dram_tensor("CF", [128, CFw], F32, kind="ExternalInput")
    out = nc.dram_tensor("out", [NPC, OUT], F32, kind="ExternalOutput")

    xl0_own = nc.dram_tensor("xl0_own", [NPC, HC], BF16)
    xl0_full = nc.dram_tensor("xl0_full", [N, HC], BF16, addr_space="Shared")
    h1T = nc.dram_tensor("h1T", [HC, NPC], BF16)
    xl1_own = nc.dram_tensor("xl1_own", [NPC, HC], BF16)
    xl1_full = nc.dram_tensor("xl1_full", [N, HC], BF16, addr_space="Shared")
    h2T = nc.dram_tensor("h2T", [HC, NPC], BF16)
    xl2_own = nc.dram_tensor("xl2_own", [NPC, 128], BF16)
    xl2_full = nc.dram_tensor("xl2_full", [N, 128], BF16, addr_space="Shared")

    rg = [list(range(NCORES))]
    qn = [0]

    def next_q():
        qn[0] = (qn[0] + 1) % 4
        return qn[0]

    with tile.TileContext(nc) as tc:
        import contextlib
        with contextlib.ExitStack() as ctx:
            cst = ctx.enter_context(tc.tile_pool(name="cst", bufs=1))
            sb = ctx.enter_context(tc.tile_pool(name="sb", bufs=3))
            gat = ctx.enter_context(tc.tile_pool(name="gat", bufs=6))
            scr = ctx.enter_context(tc.tile_pool(name="scr", bufs=2))
            ps_pxr = ctx.enter_context(tc.tile_pool(name="pspxr", bufs=2, space="PSUM"))
            ps_acc = ctx.enter_context(tc.tile_pool(name="psacc", bufs=2, space="PSUM"))
            ps_misc = ctx.enter_context(tc.tile_pool(name="psmisc", bufs=2, space="PSUM"))

            CBt = cst.tile([128, CBw], BF16)
            nc.sync.dma_start(CBt[:], CB[:])
            CFt = cst.tile([128, CFw], F32)
            nc.sync.dma_start(CFt[:], CF[:])
            idx_t = cst.tile([128, 8 * TT], I16)
            nc.sync.dma_start(idx_t[:], idx[:])
            xrA = cst.tile([128, NBLK * HC], BF16)    # L0 xr'' table
            xrB = cst.tile([128, NBLK * HC], BF16)    # L1 xr'' table
            xr2 = cst.tile([128, NBLK * OUT], BF16)   # L2 xr'' table

            def cb(name):
                r, c0, w = BCOLS[name]
                return CBt[0:r, c0:c0 + w]

            def cf(name):
                r, c0, w = FCOLS[name]
                return CFt[0:r, c0:c0 + w]

            chunks = [(i * 128, min(128, NPC - i * 128)) for i in range(NBLK)]

            # ================= phase A: L0 node tables =================
            for ci, (st, sz) in enumerate(chunks):
                xTc = sb.tile([D_IN, 128], BF16, tag="xTc")
                nc.sync.dma_start(xTc[:, :sz], xT[:, st:st + sz])
                p0 = ps_misc.tile([64, 128], F32, tag="misc")
                nc.tensor.matmul(p0[:, :sz], lhsT=cb("W_in"), rhs=xTc[:, :sz],
                                 start=True, stop=True)
                h0T = sb.tile([64, 128], BF16, tag="h0T")
                nc.scalar.activation(h0T[:, :sz], p0[:, :sz], AF.Relu,
                                     bias=cf("b_in"))
                for Wn, isl in (("Wl0f", False), ("Wr0f", True)):
                    p1 = ps_misc.tile([128, HC], F32, tag="misc")
                    nc.tensor.matmul(p1[:sz, :], lhsT=h0T[:, :sz], rhs=cb(Wn),
                                     start=True, stop=True)
                    if isl:
                        dstap = xrA[:, ci * HC:(ci + 1) * HC]
                        if sz < 128:
                            nc.vector.memset(dstap, 0.0)
                        nc.vector.tensor_copy(dstap[:sz, :], p1[:sz, :])
                    else:
                        cp = sb.tile([128, HC], BF16, tag="cp")
                        nc.vector.tensor_copy(cp[:sz, :], p1[:sz, :])
                        nc.sync.dma_start(xl0_own[st:st + sz, :], cp[:sz, :])

            nc.gpsimd.collective_compute(
                "AllGather", ALU.bypass, ins=[xl0_own[:]], outs=[xl0_full[:]],
                replica_groups=rg)

            # ================= shared edge pass (layers 0/1) =================
            def edge_pass(xl_full, xr_res, Ks, aTn, bTn, hT_dst):
                for b in range(NBLK):
                    runs = blk_runs[b]
                    T_all = sum(T for _, _, T in runs)
                    tg0 = runs[0][0]
                    st = b * BLOCK
                    nreal = min(BLOCK, NPC - st)
                    xr_ap = xr_res[:, b * HC:(b + 1) * HC]

                    ohb = sb.tile([128, Tmax * 128], F8, tag="ohb")
                    nc.sync.dma_start(ohb[:, :T_all * 128],
                                      oh8[:, tg0 * 128:(tg0 + T_all) * 128])
                    oh2b = sb.tile([128, Tmax * 128], F8, tag="oh2b")
                    nc.sync.dma_start(oh2b[:, :T_all * 128],
                                      oh28[:, tg0 * 128:(tg0 + T_all) * 128])

                    Pscr = scr.tile([128, 257 + Tmax * HC], F32, tag="scr")
                    nc.vector.memset(Pscr[:, 0:1], 0.0)

                    gts = []
                    for (tg, hlf, T) in runs:
                        g = gat.tile([128, Trun, HC], BF16, tag="g")
                        src_ap = xl_full[0:HALF, :] if hlf == 0 else xl_full[HALF:N, :]
                        k = 0
                        while k < T:
                            Tc = min(GMAX, T - k)
                            nc.gpsimd.dma_gather(
                                out_ap=g[:, k:k + Tc, :], in_ap=src_ap,
                                idxs_ap=idx_t[:, 8 * (tg + k):8 * (tg + k + Tc)],
                                num_idxs=128 * Tc, num_idxs_reg=128 * Tc,
                                elem_size=HC, queue_num=next_q())
                            k += Tc
                        gts.append((g, T))

                    # scans (leaky prefix) per group
                    tb = 0
                    for (g, T) in gts:
                        for gk in range(0, T, G):
                            Gc = min(G, T - gk)
                            pxr = ps_pxr.tile([128, G, HC], F32, tag="pxr")
                            for t in range(Gc):
                                tbt = tb + gk + t
                                nc.tensor.matmul(
                                    pxr[:, t, :],
                                    lhsT=oh2b[:, tbt * 128:(tbt + 1) * 128],
                                    rhs=xr_ap, start=True, stop=True)
                            o_ap = Pscr[:, 1 + (tb + gk) * HC:1 + (tb + gk + Gc) * HC]
                            nc.vector._custom_dve(
                                SCAN_OP,
                                out=o_ap.rearrange("p (t c) -> p t c", c=HC),
                                in0=g[:, gk:gk + Gc, :],
                                in1=pxr[:, 0:Gc, :], s0=NEG)
                        tb += T

                    # logits via boundary extraction
                    lgf = sb.tile([128, Tmax, HEADS], F32, tag="lgf")
                    for h in range(HEADS):
                        c1 = 64 * h
                        c2 = 64 * h + int(Ks[h])
                        ap2 = Pscr[:, c2:c2 + T_all * HC].rearrange(
                            "p (t o) -> p t o", o=HC)[:, :, 0]
                        ap1 = Pscr[:, c1:c1 + T_all * HC].rearrange(
                            "p (t o) -> p t o", o=HC)[:, :, 0]
                        nc.vector.scalar_tensor_tensor(
                            out=lgf[:, 0:T_all, h], in0=ap2, scalar=2.0,
                            in1=ap1, op0=ALU.mult, op1=ALU.subtract)
                    ap3 = Pscr[:, 64:64 + T_all * HC].rearrange(
                        "p (t h c) -> p t h c", h=HEADS, c=64)[:, :, :, 0]
                    nc.vector.tensor_tensor(out=lgf[:, 0:T_all, :],
                                            in0=lgf[:, 0:T_all, :], in1=ap3,
                                            op=ALU.subtract)

                    # weighted rhs (cols 0:256 = g*ex, 256:260 = ex) + scatter
                    acc = ps_acc.tile([128, 260], F32, tag="acc")
                    tb = 0
                    for (g, T) in gts:
                        for gk in range(0, T, G):
                            Gc = min(G, T - gk)
                            rhs = sb.tile([128, G, 260], BF16, tag="rhs")
                            nc.scalar.activation(rhs[:, 0:Gc, HC:HC + HEADS],
                                                 lgf[:, tb + gk:tb + gk + Gc, :],
                                                 AF.Exp)
                            nc.vector.tensor_tensor(
                                out=rhs[:, 0:Gc, 0:HC].rearrange(
                                    "p t (h c) -> p t h c", h=HEADS),
                                in0=g[:, gk:gk + Gc, :].rearrange(
                                    "p t (h c) -> p t h c", h=HEADS),
                                in1=rhs[:, 0:Gc, HC:HC + HEADS][:, :, :, None]
                                .to_broadcast([128, Gc, HEADS, 64]),
                                op=ALU.mult)
                            for t in range(Gc):
                                tbt = tb + gk + t
                                nc.tensor.matmul(
                                    acc[:], lhsT=ohb[:, tbt * 128:(tbt + 1) * 128],
                                    rhs=rhs[:, t, :],
                                    start=(tbt == 0), stop=(tbt == T_all - 1))
                        tb += T

                    # block post: normalize + BN + relu + transpose out
                    rc4 = sb.tile([128, HEADS], F32, tag="rc4")
                    nc.vector.reciprocal_approx_fast(rc4[:], acc[:, HC:HC + HEADS])
                    m1 = sb.tile([128, HC], BF16, tag="m1")
                    nc.vector.tensor_tensor(
                        out=m1[:].rearrange("p (h c) -> p h c", h=HEADS),
                        in0=acc[:, 0:HC].rearrange("p (h c) -> p h c", h=HEADS),
                        in1=rc4[:, :, None].to_broadcast([128, HEADS, 64]),
                        op=ALU.mult)
                    m2 = sb.tile([128, HC], BF16, tag="m2")
                    nc.vector.tensor_tensor(out=m2[:], in0=m1[:], in1=cb(aTn),
                                            op=ALU.mult)
                    nc.vector.tensor_tensor(out=m2[:], in0=m2[:], in1=cb(bTn),
                                            op=ALU.add)
                    h_ = sb.tile([128, HC], BF16, tag="h_")
                    nc.vector.tensor_scalar(out=h_[:], in0=m2[:], scalar1=0.0,
                                            scalar2=None, op0=ALU.max)
                    for kk in range(2):
                        tp = ps_misc.tile([128, 128], BF16, tag="misc")
                        nc.tensor.transpose(tp[:], h_[:, kk * 128:(kk + 1) * 128],
                                            cb("ident"))
                        hk = sb.tile([128, 128], BF16, tag="hk")
                        nc.scalar.copy(hk[:], tp[:])
                        nc.sync.dma_start(
                            hT_dst[kk * 128:(kk + 1) * 128, st:st + nreal],
                            hk[:, :nreal])

            edge_pass(xl0_full, xrA, K0, "aT0", "bT0", h1T)

            # ================= phase C: L1 node tables =================
            for ci, (st, sz) in enumerate(chunks):
                ht0 = sb.tile([128, 128], BF16, tag="ht0")
                nc.sync.dma_start(ht0[:, :sz], h1T[0:128, st:st + sz])
                ht1 = sb.tile([128, 128], BF16, tag="ht1")
                nc.sync.dma_start(ht1[:, :sz], h1T[128:256, st:st + sz])
                for (W0n, W1n, isl) in (("Wl1f0", "Wl1f1", False),
                                        ("Wr1f0", "Wr1f1", True)):
                    p1 = ps_misc.tile([128, HC], F32, tag="misc")
                    nc.tensor.matmul(p1[:sz, :], lhsT=ht0[:, :sz], rhs=cb(W0n),
                                     start=True, stop=False)
                    nc.tensor.matmul(p1[:sz, :], lhsT=ht1[:, :sz], rhs=cb(W1n),
                                     start=False, stop=True)
                    if isl:
                        dstap = xrB[:, ci * HC:(ci + 1) * HC]
                        if sz < 128:
                            nc.vector.memset(dstap, 0.0)
                        nc.vector.tensor_copy(dstap[:sz, :], p1[:sz, :])
                    else:
                        cp = sb.tile([128, HC], BF16, tag="cp")
                        nc.vector.tensor_copy(cp[:sz, :], p1[:sz, :])
                        nc.sync.dma_start(xl1_own[st:st + sz, :], cp[:sz, :])

            nc.gpsimd.collective_compute(
                "AllGather", ALU.bypass, ins=[xl1_own[:]], outs=[xl1_full[:]],
                replica_groups=rg)

            edge_pass(xl1_full, xrB, K1, "aT1", "bT1", h2T)

            # ================= phase E: L2 node tables =================
            for ci, (st, sz) in enumerate(chunks):
                h2t0 = sb.tile([128, 128], BF16, tag="ht0")
                nc.sync.dma_start(h2t0[:, :sz], h2T[0:128, st:st + sz])
                h2t1 = sb.tile([128, 128], BF16, tag="ht1")
                nc.sync.dma_start(h2t1[:, :sz], h2T[128:256, st:st + sz])
                h1t0 = sb.tile([128, 128], BF16, tag="h1t0")
                nc.sync.dma_start(h1t0[:, :sz], h1T[0:128, st:st + sz])
                h1t1 = sb.tile([128, 128], BF16, tag="h1t1")
                nc.sync.dma_start(h1t1[:, :sz], h1T[128:256, st:st + sz])
                for base, isl in (("Wl2f", False), ("Wr2f", True)):
                    p1 = ps_misc.tile([128, OUT], F32, tag="misc")
                    nc.tensor.matmul(p1[:sz, :], lhsT=h2t0[:, :sz],
                                     rhs=cb(base + "_h2k0"), start=True, stop=False)
                    nc.tensor.matmul(p1[:sz, :], lhsT=h2t1[:, :sz],
                                     rhs=cb(base + "_h2k1"), start=False, stop=False)
                    nc.tensor.matmul(p1[:sz, :], lhsT=h1t0[:, :sz],
                                     rhs=cb(base + "_h1k0"), start=False, stop=False)
                    nc.tensor.matmul(p1[:sz, :], lhsT=h1t1[:, :sz],
                                     rhs=cb(base + "_h1k1"), start=False, stop=True)
                    if isl:
                        dstap = xr2[:, ci * OUT:(ci + 1) * OUT]
                        if sz < 128:
                            nc.vector.memset(dstap, 0.0)
                        nc.vector.tensor_copy(dstap[:sz, :], p1[:sz, :])
                    else:
                        cp2 = sb.tile([128, 128], BF16, tag="cp2")
                        nc.vector.memset(cp2[:], 0.0)
                        nc.vector.tensor_copy(cp2[:sz, 0:OUT], p1[:sz, :])
                        nc.sync.dma_start(xl2_own[st:st + sz, :], cp2[:sz, :])

            nc.gpsimd.collective_compute(
                "AllGather", ALU.bypass, ins=[xl2_own[:]], outs=[xl2_full[:]],
                replica_groups=rg)

            # ================= phase F: L2 edge + log_softmax =================
            dlt_all = cst.tile([128, NBLK], F32)
            for b in range(NBLK):
                runs = blk_runs[b]
                T_all = sum(T for _, _, T in runs)
                tg0 = runs[0][0]
                st = b * BLOCK
                nreal = min(BLOCK, NPC - st)
                xr_ap = xr2[:, b * OUT:(b + 1) * OUT]

                ohb = sb.tile([128, Tmax * 128], F8, tag="ohb")
                nc.sync.dma_start(ohb[:, :T_all * 128],
                                  oh8[:, tg0 * 128:(tg0 + T_all) * 128])
                oh2b = sb.tile([128, Tmax * 128], F8, tag="oh2b")
                nc.sync.dma_start(oh2b[:, :T_all * 128],
                                  oh28[:, tg0 * 128:(tg0 + T_all) * 128])

                Pscr = scr.tile([128, 4 + Tmax * OUT], F32, tag="scr2")
                nc.vector.memset(Pscr[:, 0:1], 0.0)

                gts = []
                for (tg, hlf, T) in runs:
                    g = gat.tile([128, Trun, 128], BF16, tag="g2")
                    src_ap = xl2_full[0:HALF, :] if hlf == 0 else xl2_full[HALF:N, :]
                    k = 0
                    while k < T:
                        Tc = min(GMAX, T - k)
                        nc.gpsimd.dma_gather(
                            out_ap=g[:, k:k + Tc, :], in_ap=src_ap,
                            idxs_ap=idx_t[:, 8 * (tg + k):8 * (tg + k + Tc)],
                            num_idxs=128 * Tc, num_idxs_reg=128 * Tc,
                            elem_size=128, queue_num=next_q())
                        k += Tc
                    gts.append((g, T))

                tb = 0
                for (g, T) in gts:
                    for gk in range(0, T, G):
                        Gc = min(G, T - gk)
                        pxr = ps_pxr.tile([128, G, OUT], F32, tag="pxr")
                        for t in range(Gc):
                            tbt = tb + gk + t
                            nc.tensor.matmul(
                                pxr[:, t, :],
                                lhsT=oh2b[:, tbt * 128:(tbt + 1) * 128],
                                rhs=xr_ap, start=True, stop=True)
                        o_ap = Pscr[:, 1 + (tb + gk) * OUT:1 + (tb + gk + Gc) * OUT]
                        nc.vector._custom_dve(
                            SCAN_OP,
                            out=o_ap.rearrange("p (t c) -> p t c", c=OUT),
                            in0=g[:, gk:gk + Gc, 0:OUT],
                            in1=pxr[:, 0:Gc, :], s0=NEG)
                    tb += T

                lg2 = sb.tile([128, Tmax], F32, tag="lg2")
                ap2 = Pscr[:, K2:K2 + T_all * OUT].rearrange(
                    "p (t o) -> p t o", o=OUT)[:, :, 0]
                ap1 = Pscr[:, 0:T_all * OUT].rearrange(
                    "p (t o) -> p t o", o=OUT)[:, :, 0]
                nc.vector.scalar_tensor_tensor(
                    out=lg2[:, 0:T_all], in0=ap2, scalar=2.0, in1=ap1,
                    op0=ALU.mult, op1=ALU.subtract)
                ap3 = Pscr[:, OUT:OUT + T_all * OUT].rearrange(
                    "p (t o) -> p t o", o=OUT)[:, :, 0]
                nc.vector.tensor_tensor(out=lg2[:, 0:T_all], in0=lg2[:, 0:T_all],
                                        in1=ap3, op=ALU.subtract)
                rhs2 = sb.tile([128, Tmax, 3], BF16, tag="rhs2")
                nc.scalar.activation(rhs2[:, 0:T_all, 2], lg2[:, 0:T_all], AF.Exp)
                tb = 0
                for (g, T) in gts:
                    nc.vector.tensor_tensor(
                        out=rhs2[:, tb:tb + T, 0:OUT], in0=g[:, 0:T, 0:OUT],
                        in1=rhs2[:, tb:tb + T, 2:3].to_broadcast([128, T, OUT]),
                        op=ALU.mult)
                    tb += T
                acc2 = ps_acc.tile([128, 3], F32, tag="acc")
                for tbt in range(T_all):
                    nc.tensor.matmul(acc2[:],
                                     lhsT=ohb[:, tbt * 128:(tbt + 1) * 128],
                                     rhs=rhs2[:, tbt, :],
                                     start=(tbt == 0), stop=(tbt == T_all - 1))
                rc2 = sb.tile([128, 1], F32, tag="rc2")
                nc.vector.reciprocal_approx_fast(rc2[:], acc2[:, 2:3])
                v0 = sb.tile([128, 1], F32, tag="v0")
                nc.vector.tensor_scalar(out=v0[:], in0=acc2[:, RHO2[0]:RHO2[0] + 1],
                                        scalar1=float(INV_LAM2[0]), scalar2=None,
                                        op0=ALU.mult)
                d0 = sb.tile([128, 1], F32, tag="d0")
                nc.vector.scalar_tensor_tensor(
                    out=d0[:], in0=acc2[:, RHO2[1]:RHO2[1] + 1],
                    scalar=float(INV_LAM2[1]), in1=v0[:],
                    op0=ALU.mult, op1=ALU.subtract)
                nc.vector.tensor_scalar(out=dlt_all[:, b:b + 1], in0=d0[:],
                                        scalar1=rc2[:], scalar2=DB2,
                                        op0=ALU.mult, op1=ALU.add)

            # batched softplus tail: one exp + one ln for all blocks
            eall = cst.tile([128, NBLK], F32)
            nc.scalar.activation(eall[:], dlt_all[:], AF.Exp)
            nc.vector.tensor_scalar(out=eall[:], in0=eall[:], scalar1=1.0,
                                    scalar2=None, op0=ALU.add)
            lall = cst.tile([128, NBLK], F32)
            nc.scalar.activation(lall[:], eall[:], AF.Ln)
            for b in range(NBLK):
                st = b * BLOCK
                nreal = min(BLOCK, NPC - st)
                ls = sb.tile([128, 2], F32, tag="ls")
                nc.vector.tensor_scalar(out=ls[:, 0:1], in0=lall[:, b:b + 1],
                                        scalar1=-1.0, scalar2=None, op0=ALU.mult)
                nc.vector.tensor_tensor(out=ls[:, 1:2], in0=dlt_all[:, b:b + 1],
                                        in1=lall[:, b:b + 1], op=ALU.subtract)
                nc.sync.dma_start(out[st:st + nreal, :], ls[:nreal, :])

    nc.compile()
    return nc


# ---------------------------------------------------------------- entry
_CACHE = {}
_PREP_CACHE = {}
LAST_RESULTS = None


def kernel(**inputs):
    global LAST_RESULTS
    import hashlib
    x = np.asarray(inputs["x"], np.float32)
    ei = np.asarray(inputs["edge_index"]).astype(np.int64)

    f = prep_weights(inputs)
    dig = hashlib.blake2b(ei.tobytes(), digest_size=16).hexdigest()
    if dig not in _PREP_CACHE:
        _PREP_CACHE[dig] = preprocess(ei)
    idx, oh8, oh28, blk_runs, TT, Tmax, Trun, node_list = _PREP_CACHE[dig]
    CBa, BCOLS, CFa, FCOLS = pack_consts(f)

    key = (TT, Tmax, tuple(tuple(r) for rs in blk_runs for r in rs),
           tuple(f["k0"]), tuple(f["k1"]), f["k2"], CBa.shape[1], CFa.shape[1],
           tuple(f["rho2"]))
    if key not in _CACHE:
        _CACHE[key] = build(blk_runs, TT, Tmax, Trun, CBa.shape[1], CFa.shape[1],
                            BCOLS, FCOLS, f["k0"], f["k1"], f["k2"],
                            f["rho2"], f["inv_lam2"], f["db2"])
    nc = _CACHE[key]

    in_maps = []
    for c in range(NCORES):
        in_maps.append(dict(
            xT=np.ascontiguousarray(x[node_list[c]].T).astype(NPBF),
            idx=idx[c], oh8=oh8[c], oh28=oh28[c],
            CB=CBa, CF=CFa,
        ))
    res = run_bass_kernel_spmd(nc, in_maps, list(range(NCORES)))
    LAST_RESULTS = res
    full = np.empty((N, OUT), np.float32)
    for c in range(NCORES):
        full[node_list[c]] = res.results[c]["out"].astype(np.float32)
    return full
